# revision 12
# baseline (speedup 1.0000x reference)
"""Trainium2 Bass kernel for nn_MultiMPNN (gnn_message_passing).

Reference computation (B=4, N=512, Z=64, E=16, H=128):
    msgs[b,i,j,:] = z[b,i]@W_i + z[b,j]@W_j + e_feat[b,i,j]@W_e + b_msg
    agg[b,i,:]    = max_j (msgs + (adj>0 ? 0 : -inf))
    out           = z@Wu_z + agg@Wu_h + b_upd

Sharding: 8 cores = (batch b, half of destination rows i).  Each core owns
256 i-rows and the full j axis.

Device-side structure (v3):
 1. Per i-row, ONE fp8 matmul with augmented contraction K = E + Z = 80:
      lhsT_aug[80,128] = [32*W_e ; 32*W_j]                  (constant, e3m4)
      rhs_aug [80,w]   = [2*e_feat[b,i,sel].T ; 2*z[b,sel].T] (streamed, e3m4)
      PSUM[h,j] = 64*(ze + zj)  ->  max over j -> agg column
    zi + b_msg commute out of the max; they are folded into the final
    linear on the host (zit pre-scaled by 64, Wu_h divided by 64).
 2. The host compacts the j axis per row (only adj=1 columns participate);
    pad columns REPLICATE the row's first active column, which leaves the
    max unchanged and removes the -inf mask plane (K=80 fits e3m4's
    range).  e3m4 (1 byte) halves DMA vs bf16; measured rel err ~6e-3
    vs the 2e-2 gate.
 3. Drain: rows are processed in PAIRS of 4-row PSUM tiles (8 rows, one
    shared width, multiple of 16):
      T-pair: ACT stages both tiles to one bf16 SBUF tile, then a 3-level
              DVE TT-max tree (2x mode) + one short reduce_max.
      D-pair: two DVE reduce_max straight from PSUM (no ACT) — sized so
              DVE and ACT run out of work at the same time.
 4. The PE is the fastest engine but HAM-throttles to 1.2 GHz unless kept
    busy; dependency-free ldweights fillers keep its activity up.
"""

import numpy as np
import ml_dtypes

import concourse.bacc as bacc
import concourse.mybir as mybir
import concourse.tile as tile
from concourse import bass_utils
from concourse.bass_interp import get_hw_module
from contextlib import ExitStack

B, N, Z, E, H = 4, 512, 64, 16, 128
NCORES = 8
IH = N * B // NCORES          # 256 destination rows per core
KAUG = E + Z                  # 80 (no mask plane; pads replicate a real col)
RG = 4                        # rows per PSUM tile
RGW = 8                       # rows sharing one width (= one drain pair)
BANK = 512                    # f32 elems per PSUM bank
SCALE_X = 2.0                 # host scale on streamed data (e, z)
SCALE_W = 32.0                # host scale on stationary weights

# Pair-level drain schedule, rotating: 'T' = staged tree, 'D' = direct.
PAIR_PATTERN = ['T', 'T', 'T', 'T', 'D']
# Dependency-free PE keep-alive: ldweights per pair (HAM warmth).
LDW_FILL = 2

F32 = mybir.dt.float32
BF16 = mybir.dt.bfloat16
FP8 = mybir.dt.float8e3
NP_FP8 = ml_dtypes.float8_e3m4
FP8_MAX = 15.5

TRACE = False                 # test.py sets True to capture an NTFF profile
TRACE_DIR = None              # optional fixed dir for trace artifacts
LAST_RESULTS = None           # BassKernelResults of the last run (for test.py)

_MODULE_CACHE = {}


def _ensure_ntff_hook():
    """The agent image's antenv lacks axon_hooks; recreate it so
    run_bass_kernel_spmd(trace=True) can reach the axon NTFF profiler."""
    import sys
    import types

    try:
        import antenv.axon_hooks  # noqa: F401

        return
    except ImportError:
        pass
    import antenv
    from trn_agent_boot.trn_boot import _ntff_profile_via_ctypes

    state = {"h": _ntff_profile_via_ctypes("/opt/axon/libaxon_pjrt.so")}
    mod = types.ModuleType("antenv.axon_hooks")
    mod.get_axon_ntff_profile_hook = lambda: state["h"]
    mod.set_axon_ntff_profile_hook = lambda h: state.__setitem__("h", h)
    sys.modules["antenv.axon_hooks"] = mod
    antenv.axon_hooks = mod


def _build_module(widths):
    widths = list(widths)                    # one width per RGW-row pair
    row_w = [w for w in widths for _ in range(RGW)]
    offs = [0]
    for w in row_w:
        offs.append(offs[-1] + w)
    tot = offs[-1]
    nc = bacc.Bacc(
        "TRN2",
        target_bir_lowering=False,
        debug=False,
        enable_asserts=False,
        num_devices=NCORES,
    )

    stream = nc.dram_tensor("stream", [KAUG, tot], FP8, kind="ExternalInput")
    lhst = nc.dram_tensor("lhst", [KAUG, H], FP8, kind="ExternalInput")
    zit = nc.dram_tensor("zit", [H, IH], F32, kind="ExternalInput")
    hostc = nc.dram_tensor("hostc", [H, IH], F32, kind="ExternalInput")
    wuh = nc.dram_tensor("wuh", [H, H], F32, kind="ExternalInput")
    ident = nc.dram_tensor("ident", [H, H], F32, kind="ExternalInput")
    out = nc.dram_tensor("out", [IH, H], F32, kind="ExternalOutput")

    with ExitStack() as ctx:
        tc = ctx.enter_context(tile.TileContext(nc))
        const = ctx.enter_context(tc.tile_pool(name="const", bufs=1))
        mega = ctx.enter_context(tc.tile_pool(name="mega", bufs=4))
        stage_pool = ctx.enter_context(tc.tile_pool(name="stage", bufs=4))
        scr_pool = ctx.enter_context(tc.tile_pool(name="scr", bufs=6))
        psum = ctx.enter_context(tc.tile_pool(name="psum", bufs=2, space="PSUM"))

        lhst_sb = const.tile([KAUG, H], FP8, tag="lhst")
        nc.scalar.dma_start(lhst_sb[:, :], lhst.ap())
        zit_sb = const.tile([H, IH], F32, tag="zit")
        nc.scalar.dma_start(zit_sb[:, :], zit.ap())
        hostc_sb = const.tile([H, IH], F32, tag="hostc")
        nc.scalar.dma_start(hostc_sb[:, :], hostc.ap())
        wuh_sb = const.tile([H, H], F32, tag="wuh")
        nc.scalar.dma_start(wuh_sb[:, :], wuh.ap())
        ident_sb = const.tile([H, H], F32, tag="ident")
        nc.scalar.dma_start(ident_sb[:, :], ident.ap())

        magg = const.tile([H, IH], F32, tag="magg")

        # Engine warm-ups during the DMA-dominated startup window: PE HAM
        # release (sustained matmul activity) and the first ACT table load.
        warm_rhs = const.tile([KAUG, BANK], FP8, tag="warm_rhs")
        nc.vector.memset(warm_rhs[:, :], 0.0)
        warm_bf = const.tile([H, 64], BF16, tag="warm_bf")
        nc.vector.memset(warm_bf[:, :], 0.0)
        nc.scalar.copy(warm_bf[:, :64], warm_bf[:, :64])
        pw = psum.tile([H, RG * BANK], F32, tag="ps")
        for _ in range(8):
            nc.tensor.matmul(
                pw[:, :BANK], lhst_sb[:, :], warm_rhs[:, :], start=True, stop=True
            )

        # Ramp-up: small first blocks so the PE starts early; pair-aligned.
        sizes = [8, 8, 16] + [32] * 7
        assert sum(sizes) == IH

        stream_ap = stream.ap()
        row0 = 0
        paircnt = 0
        for blk, gsz in enumerate(sizes):
            belems = offs[row0 + gsz] - offs[row0]
            mb = mega.tile([KAUG, belems], FP8, tag="mega")
            nc.sync.dma_start(
                mb[:, :],
                stream_ap[:, offs[row0] : offs[row0 + gsz]],
            )
            npair = gsz // RGW
            for p in range(npair):
                w = widths[row0 // RGW + p]
                hw, qw, ow = w // 2, w // 4, w // 8
                i0 = row0 + p * RGW
                ptype = PAIR_PATTERN[paircnt % len(PAIR_PATTERN)]
                paircnt += 1

                for _ in range(LDW_FILL):
                    nc.tensor.ldweights(lhst_sb[:, :])

                # two PSUM tiles of RG=4 rows each
                tiles = []
                for t in range(2):
                    ps = psum.tile([H, RG * BANK], F32, tag="ps")
                    for r in range(RG):
                        ri = i0 + t * RG + r
                        o = offs[ri] - offs[row0]
                        nc.tensor.matmul(
                            ps[:, r * BANK : r * BANK + w],
                            lhst_sb[:, :],
                            mb[:, o : o + w],
                            start=True,
                            stop=True,
                        )
                    tiles.append(ps)

                if ptype == 'D':
                    for t in range(2):
                        ps_rows = tiles[t][:, :].rearrange(
                            "p (g j) -> p g j", g=RG
                        )
                        nc.vector.reduce_max(
                            magg[:, i0 + t * RG : i0 + (t + 1) * RG],
                            ps_rows[:, :, :w],
                            axis=mybir.AxisListType.X,
                        )
                else:
                    stage = stage_pool.tile([H, RGW * w], BF16, tag="stage")
                    st_rows = stage[:, :].rearrange("p (g j) -> p g j", g=RGW)
                    for t in range(2):
                        ps_rows = tiles[t][:, :].rearrange(
                            "p (g j) -> p g j", g=RG
                        )
                        nc.scalar.copy(
                            st_rows[:, t * RG : (t + 1) * RG, :],
                            ps_rows[:, :, :w],
                        )
                    half = scr_pool.tile([H, RGW * hw], BF16, tag="half")
                    hf = half[:, :].rearrange("p (g j) -> p g j", g=RGW)
                    nc.vector.tensor_tensor(
                        hf[:, :, :], st_rows[:, :, :hw], st_rows[:, :, hw:],
                        mybir.AluOpType.max,
                    )
                    quar = scr_pool.tile([H, RGW * qw], BF16, tag="quar")
                    qr = quar[:, :].rearrange("p (g j) -> p g j", g=RGW)
                    nc.vector.tensor_tensor(
                        qr[:, :, :], hf[:, :, :qw], hf[:, :, qw:],
                        mybir.AluOpType.max,
                    )
                    oct_ = scr_pool.tile([H, RGW * ow], BF16, tag="oct")
                    oc = oct_[:, :].rearrange("p (g j) -> p g j", g=RGW)
                    nc.vector.tensor_tensor(
                        oc[:, :, :], qr[:, :, :ow], qr[:, :, ow:],
                        mybir.AluOpType.max,
                    )
                    nc.vector.reduce_max(
                        magg[:, i0 : i0 + RGW],
                        oc[:, :, :],
                        axis=mybir.AxisListType.X,
                    )
            row0 += gsz

        aggt = const.tile([H, IH], F32, tag="aggt")
        nc.vector.tensor_add(aggt[:, :], magg[:, :], zit_sb[:, :])

        psf = psum.tile([H, RG * BANK], F32, tag="ps")
        nc.tensor.matmul(psf[:, :IH], wuh_sb[:, :], aggt[:, :], start=True, stop=True)

        outt = const.tile([H, IH], F32, tag="outt")
        nc.vector.tensor_add(outt[:, :], psf[:, :IH], hostc_sb[:, :])

        out_ap = out.ap()
        for t in range(IH // H):
            pst = psum.tile([H, RG * BANK], F32, tag="ps")
            nc.tensor.transpose(
                pst[:, :H], outt[:, t * H : (t + 1) * H], ident_sb[:, :]
            )
            osb = const.tile([H, H], F32, tag=f"osb{t}")
            nc.scalar.copy(osb[:, :], pst[:, :H])
            nc.sync.dma_start(out_ap[t * H : (t + 1) * H, :], osb[:, :])

    nc.compile()
    nc.m = get_hw_module(nc.m)
    return nc


def _fp8(x):
    return np.asarray(np.clip(x, -FP8_MAX, FP8_MAX), dtype=NP_FP8)


def _prepare(z, e_feat, adj, W_msg, b_msg, W_upd, b_upd):
    """Host-side sharding + compaction with per-pair widths.

    Rows are sorted by active-edge count (descending) so each pair of
    PSUM tiles (8 rows) gets a tight shared width (multiple of 16, for
    DVE 2x alignment through the whole tree).  Pad columns replicate the
    row's first active column (max-neutral; no mask plane needed).
    """
    W_i, W_j, W_e = W_msg[:Z], W_msg[Z : 2 * Z], W_msg[2 * Z :]
    Wu_z, Wu_h = W_upd[:Z], W_upd[Z:]

    counts = (adj > 0).sum(axis=-1)                   # [B, N]
    orders, csort = [], []
    for c in range(NCORES):
        b, half = divmod(c, NCORES // B)
        cnt = counts[b, half * IH : (half + 1) * IH]
        order = np.argsort(-cnt, kind="stable")
        orders.append(order)
        csort.append(cnt[order])
    csort = np.stack(csort)                           # [NCORES, IH]
    gmax = csort.reshape(NCORES, IH // RGW, RGW).max(-1).max(0)
    widths = np.clip((gmax + 15) // 16 * 16, 16, N).astype(int)  # [IH//RGW]
    row_w = np.repeat(widths, RGW)
    offs = np.concatenate([[0], np.cumsum(row_w)])
    tot = int(offs[-1])
    maxw = int(widths.max())

    lhst_np = _fp8(np.concatenate([W_e, W_j], axis=0) * SCALE_W)  # [80, H]
    wuh_np = np.ascontiguousarray(Wu_h / (SCALE_X * SCALE_W), np.float32)
    ident_np = np.eye(H, dtype=np.float32)

    in_maps = []
    for c in range(NCORES):
        b, half = divmod(c, NCORES // B)
        sl = slice(half * IH, (half + 1) * IH)
        order = orders[c]
        adj_blk = (adj[b, sl] > 0)[order]             # [IH, N] sorted rows
        cnt = adj_blk.sum(-1)                          # [IH]
        jorder = np.argsort(~adj_blk, axis=-1, kind="stable")[:, :maxw]
        ar = np.arange(maxw)[None, :]
        jsel = np.where(ar < cnt[:, None], jorder, jorder[:, :1])
        e_sel = np.take_along_axis(
            e_feat[b, sl][order], jsel[:, :, None], axis=1
        )                                             # [IH, maxw, E]
        z_sel = z[b][jsel]                            # [IH, maxw, Z]

        stream = np.empty((KAUG, tot), dtype=NP_FP8)
        for r in range(IH):
            w = row_w[r]
            o = offs[r]
            stream[:E, o : o + w] = _fp8(e_sel[r, :w].T * SCALE_X)
            stream[E:, o : o + w] = _fp8(z_sel[r, :w].T * SCALE_X)

        zperm = z[b, sl][order]
        in_maps.append(
            {
                "stream": stream,
                "lhst": lhst_np,
                "zit": np.ascontiguousarray(
                    ((zperm @ W_i).T + b_msg[:, None]) * (SCALE_X * SCALE_W),
                    dtype=np.float32,
                ),
                "hostc": np.ascontiguousarray(
                    (zperm @ Wu_z + b_upd).T, dtype=np.float32
                ),
                "wuh": wuh_np,
                "ident": ident_np,
            }
        )
    return in_maps, widths, orders


def kernel(z, e_feat, adj, W_msg, b_msg, W_upd, b_upd):
    global LAST_RESULTS

    z = np.asarray(z, np.float32)
    e_feat = np.asarray(e_feat, np.float32)
    adj = np.asarray(adj)
    W_msg = np.asarray(W_msg, np.float32)
    b_msg = np.asarray(b_msg, np.float32)
    W_upd = np.asarray(W_upd, np.float32)
    b_upd = np.asarray(b_upd, np.float32)

    in_maps, widths, orders = _prepare(z, e_feat, adj, W_msg, b_msg, W_upd, b_upd)

    key = tuple(widths)
    if key not in _MODULE_CACHE:
        _MODULE_CACHE[key] = _build_module(widths)
    nc = _MODULE_CACHE[key]

    if TRACE:
        _ensure_ntff_hook()
    res = bass_utils.run_bass_kernel_spmd(
        nc, in_maps, core_ids=list(range(NCORES)), trace=TRACE, tmpdir=TRACE_DIR
    )
    LAST_RESULTS = res

    full = np.empty((B, N, H), np.float32)
    for c in range(NCORES):
        b, half = divmod(c, NCORES // B)
        full[b, half * IH + orders[c]] = res.results[c]["out"]
    return full


if __name__ == "__main__":
    rng = np.random.default_rng(0)
    ins = {
        "z": rng.standard_normal((B, N, Z)).astype(np.float32),
        "e_feat": rng.standard_normal((B, N, N, E)).astype(np.float32),
        "adj": (rng.random((B, N, N)) < 0.5).astype(np.int32),
        "W_msg": (rng.standard_normal((2 * Z + E, H)) * 0.1).astype(np.float32),
        "b_msg": np.zeros(H, np.float32),
        "W_upd": (rng.standard_normal((Z + H, H)) * 0.1).astype(np.float32),
        "b_upd": np.zeros(H, np.float32),
    }
    out = kernel(**ins)
    print("out", out.shape, out.dtype, float(np.abs(out).max()))


# revision 15
# speedup vs baseline: 1.0329x; 1.0329x over previous
"""Trainium2 Bass kernel for nn_MultiMPNN (gnn_message_passing).

Reference computation (B=4, N=512, Z=64, E=16, H=128):
    msgs[b,i,j,:] = z[b,i]@W_i + z[b,j]@W_j + e_feat[b,i,j]@W_e + b_msg
    agg[b,i,:]    = max_j (msgs + (adj>0 ? 0 : -inf))
    out           = z@Wu_z + agg@Wu_h + b_upd

Sharding: 8 cores = (batch b, half of destination rows i).  Each core owns
256 i-rows and the full j axis.

Device-side structure (v3):
 1. Per i-row, ONE fp8 matmul with augmented contraction K = E + Z = 80:
      lhsT_aug[80,128] = [32*W_e ; 32*W_j]                  (constant, e3m4)
      rhs_aug [80,w]   = [2*e_feat[b,i,sel].T ; 2*z[b,sel].T] (streamed, e3m4)
      PSUM[h,j] = 64*(ze + zj)  ->  max over j -> agg column
    zi + b_msg commute out of the max; they are folded into the final
    linear on the host (zit pre-scaled by 64, Wu_h divided by 64).
 2. The host compacts the j axis per row (only adj=1 columns participate);
    pad columns REPLICATE the row's first active column, which leaves the
    max unchanged and removes the -inf mask plane (K=80 fits e3m4's
    range).  e3m4 (1 byte) halves DMA vs bf16; measured rel err ~6e-3
    vs the 2e-2 gate.
 3. Drain: rows are processed in PAIRS of 4-row PSUM tiles (8 rows, one
    shared width, multiple of 16):
      T-pair: ACT stages both tiles to one bf16 SBUF tile, then a 3-level
              DVE TT-max tree (2x mode) + one short reduce_max.
      D-pair: two DVE reduce_max straight from PSUM (no ACT) — sized so
              DVE and ACT run out of work at the same time.
 4. The PE is the fastest engine but HAM-throttles to 1.2 GHz unless kept
    busy; dependency-free ldweights fillers keep its activity up.
"""

import numpy as np
import ml_dtypes

import concourse.bacc as bacc
import concourse.mybir as mybir
import concourse.tile as tile
from concourse import bass_utils
from concourse.bass_interp import get_hw_module
from contextlib import ExitStack

B, N, Z, E, H = 4, 512, 64, 16, 128
NCORES = 8
IH = N * B // NCORES          # 256 destination rows per core
KAUG = E + Z                  # 80 (no mask plane; pads replicate a real col)
RG = 4                        # rows per PSUM tile
RGW = 8                       # rows sharing one width (= one drain pair)
BANK = 512                    # f32 elems per PSUM bank
SCALE_X = 2.0                 # host scale on streamed data (e, z)
SCALE_W = 32.0                # host scale on stationary weights

# Pair-level drain schedule, rotating: 'T' = staged tree, 'D' = direct.
PAIR_PATTERN = ['T', 'T', 'T', 'T', 'D']
# Dependency-free PE keep-alive: ldweights per pair (HAM warmth).
# Measured: fillers serialize the weight path and ADD latency — keep 0.
LDW_FILL = 0

F32 = mybir.dt.float32
BF16 = mybir.dt.bfloat16
FP8 = mybir.dt.float8e3
NP_FP8 = ml_dtypes.float8_e3m4
FP8_MAX = 15.5

TRACE = False                 # test.py sets True to capture an NTFF profile
TRACE_DIR = None              # optional fixed dir for trace artifacts
LAST_RESULTS = None           # BassKernelResults of the last run (for test.py)

_MODULE_CACHE = {}


def _ensure_ntff_hook():
    """The agent image's antenv lacks axon_hooks; recreate it so
    run_bass_kernel_spmd(trace=True) can reach the axon NTFF profiler."""
    import sys
    import types

    try:
        import antenv.axon_hooks  # noqa: F401

        return
    except ImportError:
        pass
    import antenv
    from trn_agent_boot.trn_boot import _ntff_profile_via_ctypes

    state = {"h": _ntff_profile_via_ctypes("/opt/axon/libaxon_pjrt.so")}
    mod = types.ModuleType("antenv.axon_hooks")
    mod.get_axon_ntff_profile_hook = lambda: state["h"]
    mod.set_axon_ntff_profile_hook = lambda h: state.__setitem__("h", h)
    sys.modules["antenv.axon_hooks"] = mod
    antenv.axon_hooks = mod


def _build_module(widths):
    widths = list(widths)                    # one width per RGW-row pair
    row_w = [w for w in widths for _ in range(RGW)]
    offs = [0]
    for w in row_w:
        offs.append(offs[-1] + w)
    tot = offs[-1]
    nc = bacc.Bacc(
        "TRN2",
        target_bir_lowering=False,
        debug=False,
        enable_asserts=False,
        num_devices=NCORES,
    )

    stream = nc.dram_tensor("stream", [KAUG, tot], FP8, kind="ExternalInput")
    lhst = nc.dram_tensor("lhst", [KAUG, H], FP8, kind="ExternalInput")
    zit = nc.dram_tensor("zit", [H, IH], F32, kind="ExternalInput")
    hostc = nc.dram_tensor("hostc", [H, IH], F32, kind="ExternalInput")
    wuh = nc.dram_tensor("wuh", [H, H], F32, kind="ExternalInput")
    ident = nc.dram_tensor("ident", [H, H], F32, kind="ExternalInput")
    out = nc.dram_tensor("out", [IH, H], F32, kind="ExternalOutput")

    with ExitStack() as ctx:
        tc = ctx.enter_context(tile.TileContext(nc))
        const = ctx.enter_context(tc.tile_pool(name="const", bufs=1))
        mega = ctx.enter_context(tc.tile_pool(name="mega", bufs=4))
        stage_pool = ctx.enter_context(tc.tile_pool(name="stage", bufs=4))
        scr_pool = ctx.enter_context(tc.tile_pool(name="scr", bufs=6))
        psum = ctx.enter_context(tc.tile_pool(name="psum", bufs=2, space="PSUM"))

        # Only lhst is needed before the first matmul; the tail-only consts
        # (zit/hostc/wuh/ident) are DMA'd behind it so they don't delay the
        # pipeline start.
        lhst_sb = const.tile([KAUG, H], FP8, tag="lhst")
        nc.scalar.dma_start(lhst_sb[:, :], lhst.ap())

        magg = const.tile([H, IH], F32, tag="magg")

        # Engine warm-ups during the DMA-dominated startup window: PE HAM
        # release (sustained matmul activity) and the first ACT table load.
        warm_rhs = const.tile([KAUG, BANK], FP8, tag="warm_rhs")
        nc.vector.memset(warm_rhs[:, :], 0.0)
        warm_bf = const.tile([H, 64], BF16, tag="warm_bf")
        nc.vector.memset(warm_bf[:, :], 0.0)
        nc.scalar.copy(warm_bf[:, :64], warm_bf[:, :64])
        pw = psum.tile([H, RG * BANK], F32, tag="ps")
        for _ in range(8):
            nc.tensor.matmul(
                pw[:, :BANK], lhst_sb[:, :], warm_rhs[:, :], start=True, stop=True
            )

        zit_sb = const.tile([H, IH], F32, tag="zit")
        nc.scalar.dma_start(zit_sb[:, :], zit.ap())
        hostc_sb = const.tile([H, IH], F32, tag="hostc")
        nc.scalar.dma_start(hostc_sb[:, :], hostc.ap())
        wuh_sb = const.tile([H, H], F32, tag="wuh")
        nc.scalar.dma_start(wuh_sb[:, :], wuh.ap())
        ident_sb = const.tile([H, H], F32, tag="ident")
        nc.scalar.dma_start(ident_sb[:, :], ident.ap())

        # Ramp-up: small first blocks so the PE starts early; pair-aligned.
        sizes = [8, 8, 16] + [32] * 7
        assert sum(sizes) == IH

        stream_ap = stream.ap()
        row0 = 0
        paircnt = 0
        for blk, gsz in enumerate(sizes):
            belems = offs[row0 + gsz] - offs[row0]
            mb = mega.tile([KAUG, belems], FP8, tag="mega")
            nc.sync.dma_start(
                mb[:, :],
                stream_ap[:, offs[row0] : offs[row0 + gsz]],
            )
            npair = gsz // RGW
            for p in range(npair):
                w = widths[row0 // RGW + p]
                hw, qw, ow = w // 2, w // 4, w // 8
                i0 = row0 + p * RGW
                # the last two pairs use the short direct drain so the
                # kernel tail isn't gated by a full stage+tree chain
                if paircnt >= IH // RGW - 2:
                    ptype = 'D'
                else:
                    ptype = PAIR_PATTERN[paircnt % len(PAIR_PATTERN)]
                paircnt += 1

                for _ in range(LDW_FILL):
                    nc.tensor.ldweights(lhst_sb[:, :])

                # two PSUM tiles of RG=4 rows each
                tiles = []
                for t in range(2):
                    ps = psum.tile([H, RG * BANK], F32, tag="ps")
                    for r in range(RG):
                        ri = i0 + t * RG + r
                        o = offs[ri] - offs[row0]
                        nc.tensor.matmul(
                            ps[:, r * BANK : r * BANK + w],
                            lhst_sb[:, :],
                            mb[:, o : o + w],
                            start=True,
                            stop=True,
                        )
                    tiles.append(ps)

                if ptype == 'D':
                    for t in range(2):
                        ps_rows = tiles[t][:, :].rearrange(
                            "p (g j) -> p g j", g=RG
                        )
                        nc.vector.reduce_max(
                            magg[:, i0 + t * RG : i0 + (t + 1) * RG],
                            ps_rows[:, :, :w],
                            axis=mybir.AxisListType.X,
                        )
                else:
                    stage = stage_pool.tile([H, RGW * w], BF16, tag="stage")
                    st_rows = stage[:, :].rearrange("p (g j) -> p g j", g=RGW)
                    for t in range(2):
                        ps_rows = tiles[t][:, :].rearrange(
                            "p (g j) -> p g j", g=RG
                        )
                        nc.scalar.copy(
                            st_rows[:, t * RG : (t + 1) * RG, :],
                            ps_rows[:, :, :w],
                        )
                    half = scr_pool.tile([H, RGW * hw], BF16, tag="half")
                    hf = half[:, :].rearrange("p (g j) -> p g j", g=RGW)
                    nc.vector.tensor_tensor(
                        hf[:, :, :], st_rows[:, :, :hw], st_rows[:, :, hw:],
                        mybir.AluOpType.max,
                    )
                    quar = scr_pool.tile([H, RGW * qw], BF16, tag="quar")
                    qr = quar[:, :].rearrange("p (g j) -> p g j", g=RGW)
                    nc.vector.tensor_tensor(
                        qr[:, :, :], hf[:, :, :qw], hf[:, :, qw:],
                        mybir.AluOpType.max,
                    )
                    oct_ = scr_pool.tile([H, RGW * ow], BF16, tag="oct")
                    oc = oct_[:, :].rearrange("p (g j) -> p g j", g=RGW)
                    nc.vector.tensor_tensor(
                        oc[:, :, :], qr[:, :, :ow], qr[:, :, ow:],
                        mybir.AluOpType.max,
                    )
                    nc.vector.reduce_max(
                        magg[:, i0 : i0 + RGW],
                        oc[:, :, :],
                        axis=mybir.AxisListType.X,
                    )
            row0 += gsz

        aggt = const.tile([H, IH], F32, tag="aggt")
        nc.vector.tensor_add(aggt[:, :], magg[:, :], zit_sb[:, :])

        psf = psum.tile([H, RG * BANK], F32, tag="ps")
        nc.tensor.matmul(psf[:, :IH], wuh_sb[:, :], aggt[:, :], start=True, stop=True)

        outt = const.tile([H, IH], F32, tag="outt")
        nc.vector.tensor_add(outt[:, :], psf[:, :IH], hostc_sb[:, :])

        out_ap = out.ap()
        for t in range(IH // H):
            pst = psum.tile([H, RG * BANK], F32, tag="ps")
            nc.tensor.transpose(
                pst[:, :H], outt[:, t * H : (t + 1) * H], ident_sb[:, :]
            )
            osb = const.tile([H, H], F32, tag=f"osb{t}")
            nc.scalar.copy(osb[:, :], pst[:, :H])
            nc.sync.dma_start(out_ap[t * H : (t + 1) * H, :], osb[:, :])

    nc.compile()
    nc.m = get_hw_module(nc.m)
    return nc


def _fp8(x):
    return np.asarray(np.clip(x, -FP8_MAX, FP8_MAX), dtype=NP_FP8)


def _prepare(z, e_feat, adj, W_msg, b_msg, W_upd, b_upd):
    """Host-side sharding + compaction with per-pair widths.

    Rows are sorted by active-edge count (descending) so each pair of
    PSUM tiles (8 rows) gets a tight shared width (multiple of 16, for
    DVE 2x alignment through the whole tree).  Pad columns replicate the
    row's first active column (max-neutral; no mask plane needed).
    """
    W_i, W_j, W_e = W_msg[:Z], W_msg[Z : 2 * Z], W_msg[2 * Z :]
    Wu_z, Wu_h = W_upd[:Z], W_upd[Z:]

    counts = (adj > 0).sum(axis=-1)                   # [B, N]
    orders, csort = [], []
    for c in range(NCORES):
        b, half = divmod(c, NCORES // B)
        cnt = counts[b, half * IH : (half + 1) * IH]
        order = np.argsort(-cnt, kind="stable")
        orders.append(order)
        csort.append(cnt[order])
    csort = np.stack(csort)                           # [NCORES, IH]
    gmax = csort.reshape(NCORES, IH // RGW, RGW).max(-1).max(0)
    widths = np.clip((gmax + 15) // 16 * 16, 16, N).astype(int)  # [IH//RGW]
    row_w = np.repeat(widths, RGW)
    offs = np.concatenate([[0], np.cumsum(row_w)])
    tot = int(offs[-1])
    maxw = int(widths.max())

    lhst_np = _fp8(np.concatenate([W_e, W_j], axis=0) * SCALE_W)  # [80, H]
    wuh_np = np.ascontiguousarray(Wu_h / (SCALE_X * SCALE_W), np.float32)
    ident_np = np.eye(H, dtype=np.float32)

    in_maps = []
    for c in range(NCORES):
        b, half = divmod(c, NCORES // B)
        sl = slice(half * IH, (half + 1) * IH)
        order = orders[c]
        adj_blk = (adj[b, sl] > 0)[order]             # [IH, N] sorted rows
        cnt = adj_blk.sum(-1)                          # [IH]
        jorder = np.argsort(~adj_blk, axis=-1, kind="stable")[:, :maxw]
        ar = np.arange(maxw)[None, :]
        jsel = np.where(ar < cnt[:, None], jorder, jorder[:, :1])
        e_sel = np.take_along_axis(
            e_feat[b, sl][order], jsel[:, :, None], axis=1
        )                                             # [IH, maxw, E]
        z_sel = z[b][jsel]                            # [IH, maxw, Z]

        stream = np.empty((KAUG, tot), dtype=NP_FP8)
        for r in range(IH):
            w = row_w[r]
            o = offs[r]
            stream[:E, o : o + w] = _fp8(e_sel[r, :w].T * SCALE_X)
            stream[E:, o : o + w] = _fp8(z_sel[r, :w].T * SCALE_X)

        zperm = z[b, sl][order]
        in_maps.append(
            {
                "stream": stream,
                "lhst": lhst_np,
                "zit": np.ascontiguousarray(
                    ((zperm @ W_i).T + b_msg[:, None]) * (SCALE_X * SCALE_W),
                    dtype=np.float32,
                ),
                "hostc": np.ascontiguousarray(
                    (zperm @ Wu_z + b_upd).T, dtype=np.float32
                ),
                "wuh": wuh_np,
                "ident": ident_np,
            }
        )
    return in_maps, widths, orders


def kernel(z, e_feat, adj, W_msg, b_msg, W_upd, b_upd):
    global LAST_RESULTS

    z = np.asarray(z, np.float32)
    e_feat = np.asarray(e_feat, np.float32)
    adj = np.asarray(adj)
    W_msg = np.asarray(W_msg, np.float32)
    b_msg = np.asarray(b_msg, np.float32)
    W_upd = np.asarray(W_upd, np.float32)
    b_upd = np.asarray(b_upd, np.float32)

    in_maps, widths, orders = _prepare(z, e_feat, adj, W_msg, b_msg, W_upd, b_upd)

    key = tuple(widths)
    if key not in _MODULE_CACHE:
        _MODULE_CACHE[key] = _build_module(widths)
    nc = _MODULE_CACHE[key]

    if TRACE:
        _ensure_ntff_hook()
    res = bass_utils.run_bass_kernel_spmd(
        nc, in_maps, core_ids=list(range(NCORES)), trace=TRACE, tmpdir=TRACE_DIR
    )
    LAST_RESULTS = res

    full = np.empty((B, N, H), np.float32)
    for c in range(NCORES):
        b, half = divmod(c, NCORES // B)
        full[b, half * IH + orders[c]] = res.results[c]["out"]
    return full


if __name__ == "__main__":
    rng = np.random.default_rng(0)
    ins = {
        "z": rng.standard_normal((B, N, Z)).astype(np.float32),
        "e_feat": rng.standard_normal((B, N, N, E)).astype(np.float32),
        "adj": (rng.random((B, N, N)) < 0.5).astype(np.int32),
        "W_msg": (rng.standard_normal((2 * Z + E, H)) * 0.1).astype(np.float32),
        "b_msg": np.zeros(H, np.float32),
        "W_upd": (rng.standard_normal((Z + H, H)) * 0.1).astype(np.float32),
        "b_upd": np.zeros(H, np.float32),
    }
    out = kernel(**ins)
    print("out", out.shape, out.dtype, float(np.abs(out).max()))
